# revision 19
# baseline (speedup 1.0000x reference)
"""Causal self-attention (B=4, T=2048, C=1024, H=16) on 8 Trainium2 cores.

Sharding: core i handles batch b = i//2 and head-group g = i%2 (8 heads,
512 channels). The host pre-transposes x (so the kernel consumes xT and
spends no PE cycles on transposes) and pre-slices the weights per core.

Kernel structure (single fused pipeline):
  quarter Q in 0..3:  QKV matmuls for T-columns [512Q, 512Q+512)
                      interleaved (Q>=1) with attention chunk Q-1
  attention chunk 3 runs interleaved with projections of chunks 0-2
  projection of chunk 3 drains last (via the Activation engine).

Attention per (chunk c, head l): scores via f32r matmuls into PSUM,
exp on ACT -> e (bf16), causal masking applied in-place on e by GPSIMD
affine_select (only the 128-col diagonal triangles), then att@v with a
fused ones-column producing softmax denominators.

Bias folding (exact):
  - k-bias cancels in softmax (dropped).
  - v-bias contributes (b_v @ W_proj), folded into the host bias row.
  - q-bias added by DVE (tensor_scalar_add) when draining q from PSUM.
"""

import sys
from contextlib import ExitStack

import ml_dtypes
import numpy as np

BF16NP = ml_dtypes.bfloat16

sys.path.insert(0, "/opt/trn_rl_repo")

import concourse.bass as bass  # noqa: E402
import concourse.mybir as mybir  # noqa: E402
from concourse.masks import make_identity  # noqa: E402
from concourse.tile import TileContext  # noqa: E402

F32 = mybir.dt.float32
F32R = mybir.dt.float32r
BF16 = mybir.dt.bfloat16
EXP = mybir.ActivationFunctionType.Exp
IS_GE = mybir.AluOpType.is_ge

B, T, C, H, D = 4, 2048, 1024, 16, 64
NCORES = 8
HL = 8          # heads per core
CL = HL * D     # 512 local channels
KC = 8          # contraction chunks (1024 / 128)
SCALE = 1.0 / 8.0  # 1/sqrt(64)


# --------------------------------------------------------------------------
# Workaround: this walrus build accepts only ONE sync-wait per instruction.
# Tile emits several (operand deps, tail drain). Split extras onto fresh
# single-wait EventSemaphore instructions just before each offender on the
# same engine — semantics unchanged, the sequencer blocks on each in turn.
# --------------------------------------------------------------------------
def _split_multiwait_insts(nc):
    ctr = 0
    for f in nc.m.functions:
        for blk in f.blocks:
            insts = list(blk.instructions)
            new_list = []
            changed = False
            for inst in insts:
                si = inst.sync_info
                if si is not None and len(si.on_wait) > 1:
                    waits = list(si.on_wait)
                    keep_idx = len(waits) - 1
                    for i, w in enumerate(waits):
                        if w.wait_reg is not None:
                            keep_idx = i
                            break
                    for i, w in enumerate(waits):
                        if i == keep_idx:
                            continue
                        ev = mybir.InstEventSemaphore(
                            name=f"evsplit_{ctr}", ins=[], outs=[]
                        )
                        ctr += 1
                        ev.engine = inst.engine
                        ev.sync_info = mybir.SyncInfo(on_wait=[w], on_update=[])
                        new_list.append(ev)
                    inst.sync_info.on_wait = [waits[keep_idx]]
                    changed = True
                new_list.append(inst)
            if changed:
                blk.instructions = new_list


def build_bass(repeat=1):
    nc = bass.Bass("TRN2", target_bir_lowering=False, debug=False)

    xT_d = nc.dram_tensor("xT", [C, T], BF16, kind="ExternalInput")
    wqk_d = nc.dram_tensor("wqk", [C, 2 * CL], BF16, kind="ExternalInput")
    wv_d = nc.dram_tensor("wv", [C, CL], BF16, kind="ExternalInput")
    bq_d = nc.dram_tensor("bq", [4, 128, 1], F32, kind="ExternalInput")
    wp_d = nc.dram_tensor("wp", [CL, C], BF16, kind="ExternalInput")
    out_d = nc.dram_tensor("out", [T, C], F32, kind="ExternalOutput")
    scr_d = nc.dram_tensor("scr", [4 * HL, 512], F32)  # recip bounce buffer

    with TileContext(nc) as tc:
        for _rep in range(repeat):
            _emit_body(nc, tc, xT_d, wqk_d, wv_d, bq_d, wp_d, out_d, scr_d)

    _split_multiwait_insts(nc)
    return nc


def _emit_body(nc, tc, xT_d, wqk_d, wv_d, bq_d, wp_d, out_d, scr_d):
    with ExitStack() as ctx:
        const = ctx.enter_context(tc.tile_pool(name="const", bufs=1))
        big = ctx.enter_context(tc.tile_pool(name="big", bufs=1))

        # persistent tensors
        qkT = big.tile([128, KC, T], F32R)      # dim1: 0-3 q m-tiles, 4-7 k
        vaug = big.tile([128, 16, HL * 65], BF16)  # per T-block: 8x(64 v + 1)
        yT = big.tile([128, 4, T], BF16)        # local channels x T

        bq_sb = const.tile([128, 4, 1], F32)
        nc.sync.dma_start(out=bq_sb, in_=bq_d.ap().rearrange("a p o -> p a o"))

        e_ctx = ctx.enter_context(tc.tile_pool(name="epool", bufs=4))
        rc_pool = ctx.enter_context(tc.tile_pool(name="rcpool", bufs=3))
        ynat_pool = ctx.enter_context(tc.tile_pool(name="ynat", bufs=2))
        out_pool = ctx.enter_context(tc.tile_pool(name="opool", bufs=2))
        qk_ps = ctx.enter_context(
            tc.tile_pool(name="qkps", bufs=2, space="PSUM")
        )
        y_ps = ctx.enter_context(tc.tile_pool(name="yps", bufs=2, space="PSUM"))

        idn = const.tile([128, 128], F32)
        make_identity(nc, idn)
        # additive causal masks: 0 on/above valid region, -30000 below
        tri_neg = const.tile([128, 128], BF16)
        nc.vector.memset(tri_neg, 0.0)
        nc.gpsimd.affine_select(
            out=tri_neg, in_=tri_neg, pattern=[[1, 128]],
            compare_op=IS_GE, fill=-30000.0, base=0, channel_multiplier=-1,
        )
        m2_neg = const.tile([128, 256], BF16)
        nc.vector.memset(m2_neg, 0.0)
        nc.gpsimd.affine_select(
            out=m2_neg, in_=m2_neg, pattern=[[1, 256]],
            compare_op=IS_GE, fill=-30000.0, base=-128, channel_multiplier=-1,
        )

        # ------------------------------------------------------------------
        # attention emission units for one (chunk c, head l) group
        # ------------------------------------------------------------------
        def att_group_units(c, l, ynat):
            """Yield emit-closures: scores/exp/mask for pair pj, then avs
            (software-pipelined one pair behind), then finalize."""
            row = (l % 2) * 64
            qtile = l // 2
            ktile = 4 + l // 2
            npairs = 2 * c + 2
            state = {"yps": None, "pairs": [None] * npairs}

            def make_scores(pj):
                def emit():
                    if state["yps"] is None:
                        state["yps"] = y_ps.tile([128, 4, 65], F32, tag="yps", name="yps")
                    j0 = 2 * pj - 4 * c
                    pqk = qk_ps.tile([128, 1024], F32, tag="qk")
                    e = e_ctx.tile([128, 1024], BF16, tag="e")
                    # per-half write windows (multiples of 128, >=256 wide
                    # except the 384 case, to keep f32r at full rate)
                    if j0 == 0:
                        los = (0, 128)
                    elif j0 == 2:
                        los = (256, 256)
                    else:
                        los = (0, 0)
                    for h in range(2):
                        tk = 2 * pj + h
                        lo = los[h]
                        nc.tensor.matmul(
                            out=pqk[:, h * 512 + lo:(h + 1) * 512],
                            lhsT=qkT[row:row + 64, ktile,
                                     tk * 128:(tk + 1) * 128],
                            rhs=qkT[row:row + 64, qtile,
                                    c * 512 + lo:(c + 1) * 512],
                            start=True,
                            stop=True,
                        )
                    # additive causal masks on the diagonal PSUM slices
                    # (before exp, so av depends on the exp alone)
                    if j0 == 0:
                        nc.vector.tensor_add(
                            pqk[:, 0:128], pqk[:, 0:128], tri_neg
                        )
                        nc.vector.tensor_add(
                            pqk[:, 640:768], pqk[:, 640:768], tri_neg
                        )
                    elif j0 == 2:
                        nc.vector.tensor_add(
                            pqk[:, 256:384], pqk[:, 256:384], tri_neg
                        )
                        nc.vector.tensor_add(
                            pqk[:, 768:1024], pqk[:, 768:1024], m2_neg
                        )
                    # single exp over the written union (interior stale
                    # regions are exp'd but never read by the avs)
                    lo0 = los[0]
                    nc.scalar.activation(
                        out=e[:, lo0:1024], in_=pqk[:, lo0:1024],
                        func=EXP, scale=SCALE,
                    )
                    state["pairs"][pj] = (e, los)
                return emit

            def make_av(pj):
                def emit():
                    e, los = state["pairs"][pj]
                    yps = state["yps"]
                    for h in range(2):
                        tk = 2 * pj + h
                        for qb in range(4):
                            if tk > 4 * c + qb:
                                continue  # above the causal diagonal
                            nc.tensor.matmul(
                                out=yps[:, qb, :],
                                lhsT=e[:, h * 512 + qb * 128:
                                       h * 512 + (qb + 1) * 128],
                                rhs=vaug[:, tk, l * 65:(l + 1) * 65],
                                start=(tk == 0),
                                stop=(tk == 4 * c + qb),
                            )
                    state["pairs"][pj] = None
                return emit

            def finalize():
                yps = state["yps"]
                rcp = rc_pool.tile([128, 4, 1], F32, tag="rc")
                nc.vector.reciprocal(out=rcp, in_=yps[:, :, 64:65])
                for qb in range(4):
                    nc.vector.tensor_scalar_mul(
                        out=ynat[:, qb, l, :],
                        in0=yps[:, qb, 0:64],
                        scalar1=rcp[:, qb, :],
                    )

            # software-pipelined unit order: sc0 sc1 av0 sc2 av1 ... av(n-1)
            yield make_scores(0)
            yield make_scores(1)
            for pj in range(2, npairs):
                yield make_av(pj - 2)
                yield make_scores(pj)
            yield make_av(npairs - 2)
            yield make_av(npairs - 1)
            yield finalize  # deferred by the caller's unit interleaving

        def att_chunk_units(c, trans_ref):
            ynat = ynat_pool.tile(
                [128, 4, HL, 64], F32, tag="ynat", name="ynat"
            )

            def transpose_cb(cb):
                def emit():
                    ynf = ynat.rearrange("p q l d -> p q (l d)")
                    tp = trans_ref["pool"].tile(
                        [128, 512], F32, tag=trans_ref["tag"], name="ytr"
                    )
                    for qb in range(4):
                        nc.tensor.transpose(
                            out=tp[:, qb * 128:(qb + 1) * 128],
                            in_=ynf[:, qb, cb * 128:(cb + 1) * 128],
                            identity=idn,
                        )
                    nc.vector.tensor_copy(
                        out=yT[:, cb, c * 512:(c + 1) * 512], in_=tp
                    )
                return emit

            # defer each group's finalize into the next group's stream (so
            # its DVE ops never head-of-line-block the DVE queue), and each
            # channel-block transpose by one further group
            units = []
            pending = []  # deferred units with countdowns
            for l in range(HL):
                gu = list(att_group_units(c, l, ynat))
                body, fin = gu[:-1], gu[-1]
                for i, u in enumerate(body):
                    units.append(u)
                    if pending and i == 1:
                        units.extend(pending)
                        pending = []
                pending.append(fin)
                if l % 2 == 1 and l >= 3:
                    pending.append(transpose_cb((l - 2) // 2))
            units.extend(pending)
            units.append(transpose_cb(3))
            return units

        def merge_ratio(a, b):
            """Proportionally interleave two unit lists."""
            out, bi = [], 0
            na, nb = len(a), len(b)
            for i, u in enumerate(a):
                out.append(u)
                want = (i + 1) * nb // na
                while bi < want:
                    out.append(b[bi])
                    bi += 1
            out.extend(b[bi:])
            return out

        # ------------------------------------------------------------------
        # Phase A scoped pools: x staging + qkv weights + qkv PSUM
        # ------------------------------------------------------------------
        with tc.tile_pool(name="xstage", bufs=2) as xstage, \
             tc.tile_pool(name="wqkv", bufs=1) as wqkv_pool, \
             tc.tile_pool(name="mmps", bufs=2, space="PSUM") as mm_ps:

            wqk_sb = wqkv_pool.tile([128, KC, 2 * CL], BF16)
            wv_sb = wqkv_pool.tile([128, KC, CL], BF16)
            # per-m-column-slab weight loads on the ACT queue
            for m in range(8):
                nc.scalar.dma_start(
                    out=wqk_sb[:, :, m * 128:(m + 1) * 128],
                    in_=wqk_d.ap().rearrange("(k p) c -> p k c", p=128)[
                        :, :, m * 128:(m + 1) * 128
                    ],
                )
            for m in range(4):
                nc.scalar.dma_start(
                    out=wv_sb[:, :, m * 128:(m + 1) * 128],
                    in_=wv_d.ap().rearrange("(k p) c -> p k c", p=128)[
                        :, :, m * 128:(m + 1) * 128
                    ],
                )

            xt_view = xT_d.ap().rearrange("(k p) t -> p k t", p=128)

            def qkv_tile_units(quarter):
                """12 emit-closures: 8 qk m-tiles + 4 v t-blocks, each fed by
                two half-quarter x stages."""
                halves = [None, None]

                def load_half(hh):
                    def emit():
                        xs = xstage.tile([128, KC, 256], BF16, tag="xq")
                        base = quarter * 512 + hh * 256
                        nc.sync.dma_start(
                            out=xs, in_=xt_view[:, :, base:base + 256]
                        )
                        halves[hh] = xs
                    return emit

                def qk_tile(m):
                    def emit():
                        pq = mm_ps.tile([128, 512], F32, tag="mm")
                        for hh in range(2):
                            xs = halves[hh]
                            for k in range(KC):
                                nc.tensor.matmul(
                                    out=pq[:, hh * 256:(hh + 1) * 256],
                                    lhsT=wqk_sb[:, k, m * 128:(m + 1) * 128],
                                    rhs=xs[:, k, :],
                                    start=(k == 0),
                                    stop=(k == KC - 1),
                                )
                        dst = qkT[:, m, quarter * 512:(quarter + 1) * 512]
                        if m < 4:  # q: add per-partition q-bias
                            nc.vector.tensor_scalar_add(
                                out=dst, in0=pq, scalar1=bq_sb[:, m, :]
                            )
                        else:
                            nc.vector.tensor_copy(out=dst, in_=pq)
                    return emit

                def v_tile(tt):
                    def emit():
                        hh = tt // 2
                        xs = halves[hh]
                        sub = tt % 2
                        pv = mm_ps.tile([128, 512], F32, tag="mm")
                        for k in range(KC):
                            nc.tensor.matmul(
                                out=pv,
                                lhsT=xs[:, k, sub * 128:(sub + 1) * 128],
                                rhs=wv_sb[:, k, :],
                                start=(k == 0),
                                stop=(k == KC - 1),
                            )
                        tglob = quarter * 4 + tt
                        nc.vector.memset(vaug[:, tglob, :], 1.0)
                        nc.vector.tensor_copy(
                            out=vaug[:, tglob, :].rearrange(
                                "p (h c) -> p h c", c=65
                            )[:, :, 0:64],
                            in_=pv.rearrange("p (h c) -> p h c", c=64),
                        )
                    return emit

                units = [load_half(0), load_half(1)]
                for m in range(8):  # same order as the weight-slab DMAs
                    units.append(qk_tile(m))
                for tt in range(4):
                    units.append(v_tile(tt))
                return units

            def weave(primary, filler):
                """Emit primary units, interspersing filler units evenly."""
                np_, nf = len(primary), len(filler)
                fi = 0
                for i, u in enumerate(primary):
                    u()
                    want = (i + 1) * nf // np_
                    while fi < want:
                        filler[fi]()
                        fi += 1
                while fi < nf:
                    filler[fi]()
                    fi += 1

            trans_ref = {"pool": mm_ps, "tag": "mm"}
            # win0: qkv quarter 0 alone
            for u in qkv_tile_units(0):
                u()
            # win1..2: qkv quarter Q woven with attention chunk Q-1
            for quarter in range(1, 3):
                weave(att_chunk_units(quarter - 1, trans_ref),
                      qkv_tile_units(quarter))
            # win3: only the first half of chunk 2 fits qkv3's span (ACT
            # pacing) - the rest spills into the chunk-3 region
            att2 = att_chunk_units(2, trans_ref)
            cut = len(att2) // 2
            weave(att2[:cut], qkv_tile_units(3))
            att2_rest = att2[cut:]

        # ------------------------------------------------------------------
        # win4: attention chunk 3 woven with projections of chunks 0-2;
        # then chunk-3 projections drain via ACT.
        # ------------------------------------------------------------------
        with tc.tile_pool(name="wp", bufs=1) as wp_pool, \
             tc.tile_pool(name="projps", bufs=2, space="PSUM") as proj_ps:

            wp_sb = wp_pool.tile([128, 4, C], BF16)
            for k in range(4):
                nc.scalar.dma_start(
                    out=wp_sb[:, k, :],
                    in_=wp_d.ap().rearrange("(k p) c -> p k c", p=128)[:, k, :],
                )

            def proj_units(tq, act=False):
                os_ = {}

                def piece(oc):
                    def emit():
                        if oc == 0:
                            os_["t"] = out_pool.tile([128, 1024], F32, tag="os", name="os")
                        pp = proj_ps.tile([128, 512], F32, tag="proj")
                        for k in range(4):
                            nc.tensor.matmul(
                                out=pp,
                                lhsT=yT[:, k, tq * 128:(tq + 1) * 128],
                                rhs=wp_sb[:, k, oc * 512:(oc + 1) * 512],
                                start=(k == 0),
                                stop=(k == 3),
                            )
                        dst = os_["t"][:, oc * 512:(oc + 1) * 512]
                        if act:
                            nc.scalar.copy(out=dst, in_=pp)
                        else:
                            nc.vector.tensor_copy(out=dst, in_=pp)
                        if oc == 1:
                            nc.sync.dma_start(
                                out=out_d.ap()[tq * 128:(tq + 1) * 128, :],
                                in_=os_["t"],
                            )
                    return emit

                return [piece(0), piece(1)]

            fillers = []
            for tq in range(12):  # projections of chunks 0-2
                fillers.extend(proj_units(tq))

            def weave2(primary, filler):
                np_, nf = len(primary), len(filler)
                fi = 0
                for i, u in enumerate(primary):
                    u()
                    want = (i + 1) * nf // np_
                    while fi < want:
                        filler[fi]()
                        fi += 1
                while fi < nf:
                    filler[fi]()
                    fi += 1

            trans_ref["pool"] = proj_ps
            trans_ref["tag"] = "proj"
            att3 = att_chunk_units(3, trans_ref)
            merged = merge_ratio(att2_rest, att3)
            # hold projections until wp has had time to load (the wp DMA can
            # only start once the qkv weight pool is freed at win3's end)
            hold = 16
            for u in merged[:hold]:
                u()
            weave2(merged[hold:], fillers)
            for tq in range(12, 16):  # chunk-3 projections drain via ACT
                for u in proj_units(tq, act=True):
                    u()


# --------------------------------------------------------------------------
# Cached PJRT execution (mirrors bass2jax.run_bass_via_pjrt but reuses the
# compiled executable across kernel() calls).
# --------------------------------------------------------------------------
_CACHE = {}


def _get_runner(repeat=1):
    key = ("runner", repeat)
    if key in _CACHE:
        return _CACHE[key]

    import jax
    from jax.sharding import Mesh, PartitionSpec
    from jax.experimental.shard_map import shard_map
    from concourse import bass2jax

    nc = build_bass(repeat=repeat)
    bass2jax.install_neuronx_cc_hook()

    partition_name = (
        nc.partition_id_tensor.name if nc.partition_id_tensor else None
    )
    in_names, out_names, out_avals, zero_shapes = [], [], [], []
    for alloc in nc.m.functions[0].allocations:
        if not isinstance(alloc, mybir.MemoryLocationSet):
            continue
        name = alloc.memorylocations[0].name
        if alloc.kind == "ExternalInput":
            if name != partition_name:
                in_names.append(name)
        elif alloc.kind == "ExternalOutput":
            shape = tuple(alloc.tensor_shape)
            dtype = mybir.dt.np(alloc.dtype)
            out_names.append(name)
            out_avals.append(jax.core.ShapedArray(shape, dtype))
            zero_shapes.append((shape, dtype))
    n_params = len(in_names)
    n_outs = len(out_avals)
    all_in_names = list(in_names) + list(out_names)
    if partition_name is not None:
        all_in_names.append(partition_name)

    def _body(*args):
        operands = list(args)
        if partition_name is not None:
            operands.append(bass2jax.partition_id_tensor())
        outs = bass2jax._bass_exec_p.bind(
            *operands,
            out_avals=tuple(out_avals),
            in_names=tuple(all_in_names),
            out_names=tuple(out_names),
            lowering_input_output_aliases=(),
            sim_require_finite=True,
            sim_require_nnan=True,
            nc=nc,
        )
        return tuple(outs)

    devices = jax.devices()[:NCORES]
    mesh = Mesh(np.asarray(devices), ("core",))
    in_specs = (PartitionSpec("core"),) * (n_params + n_outs)
    out_specs = (PartitionSpec("core"),) * n_outs
    donate = tuple(range(n_params, n_params + n_outs))
    sharded = jax.jit(
        shard_map(
            _body, mesh=mesh, in_specs=in_specs, out_specs=out_specs,
            check_rep=False,
        ),
        donate_argnums=donate,
        keep_unused=True,
    )

    runner = {
        "sharded": sharded,
        "in_names": in_names,
        "out_names": out_names,
        "zero_shapes": zero_shapes,
        "n_params": n_params,
        "mesh": mesh,
    }
    _CACHE[key] = runner
    return runner


def _make_core_inputs(x, W_attn, b_attn, W_proj):
    """Per-core input dicts (core i: batch i//2, head-group i%2)."""
    x = np.ascontiguousarray(x, dtype=np.float32)
    W_attn = np.ascontiguousarray(W_attn, dtype=np.float32)
    b_attn = np.ascontiguousarray(b_attn, dtype=np.float32)
    W_proj = np.ascontiguousarray(W_proj, dtype=np.float32)

    per_group = []
    for g in range(2):
        s = g * CL
        wqk = np.ascontiguousarray(
            np.concatenate(
                [W_attn[:, s:s + CL], W_attn[:, C + s:C + s + CL]], axis=1
            ).astype(BF16NP)
        )
        wv = np.ascontiguousarray(
            W_attn[:, 2 * C + s:2 * C + s + CL].astype(BF16NP)
        )
        bq = np.ascontiguousarray(b_attn[s:s + CL].reshape(4, 128, 1))
        wp = np.ascontiguousarray(W_proj[s:s + CL, :].astype(BF16NP))
        per_group.append((wqk, wv, bq, wp))

    xts = [np.ascontiguousarray(x[b].T.astype(BF16NP)) for b in range(B)]

    in_maps = []
    for core in range(NCORES):
        b_i, g = core // 2, core % 2
        wqk, wv, bq, wp = per_group[g]
        in_maps.append(
            {"xT": xts[b_i], "wqk": wqk, "wv": wv, "bq": bq, "wp": wp}
        )
    return in_maps


def run_cores(in_maps, timing_reps=0, repeat=1):
    """Run the SPMD kernel. Returns (list of per-core output dicts, best_ns).

    timing_reps > 0 additionally re-executes the cached executable on
    device-resident inputs and reports the best wall-clock per call in ns.
    """
    import jax, time

    r = _get_runner(repeat=repeat)
    per_core = [
        [np.asarray(m[name]) for name in r["in_names"]] for m in in_maps
    ]
    concat_in = [
        np.concatenate([per_core[c][i] for c in range(NCORES)], axis=0)
        for i in range(len(r["in_names"]))
    ]
    def zeros():
        return [
            np.zeros((NCORES * s[0], *s[1:]), dt) for (s, dt) in r["zero_shapes"]
        ]

    out_arrs = r["sharded"](*concat_in, *zeros())
    outs_np = [np.asarray(a) for a in out_arrs]

    best_ns = None
    if timing_reps > 0:
        from jax.sharding import NamedSharding, PartitionSpec

        shard = NamedSharding(r["mesh"], PartitionSpec("core"))
        dev_in = [jax.device_put(a, shard) for a in concat_in]
        for a in dev_in:
            a.block_until_ready()
        # pre-stage one donated zero-set per timed call (donation consumes them)
        zsets = []
        for _ in range(timing_reps + 1):
            zs = [jax.device_put(z, shard) for z in zeros()]
            for a in zs:
                a.block_until_ready()
            zsets.append(zs)
        res = r["sharded"](*dev_in, *zsets[0])  # warm
        for a in res:
            a.block_until_ready()
        times = []
        for i in range(timing_reps):
            t0 = time.perf_counter()
            res = r["sharded"](*dev_in, *zsets[i + 1])
            for a in res:
                a.block_until_ready()
            t1 = time.perf_counter()
            times.append(t1 - t0)
        best_ns = int(min(times) * 1e9)

    results = []
    for c in range(NCORES):
        m = {}
        for i, name in enumerate(r["out_names"]):
            full = outs_np[i]
            shape = r["zero_shapes"][i][0]
            m[name] = full.reshape(NCORES, *shape)[c]
        results.append(m)
    return results, best_ns


def kernel(x, W_attn, b_attn, W_proj, b_proj, _timing_reps=0, _return_ns=False):
    x = np.asarray(x, dtype=np.float32)
    W_attn = np.asarray(W_attn, dtype=np.float32)
    b_attn = np.asarray(b_attn, dtype=np.float32)
    W_proj = np.asarray(W_proj, dtype=np.float32)
    b_proj = np.asarray(b_proj, dtype=np.float32)

    in_maps = _make_core_inputs(x, W_attn, b_attn, W_proj)
    results, best_ns = run_cores(in_maps, timing_reps=_timing_reps)

    # v-bias contributes a constant row through the projection
    bias_row = (b_proj + b_attn[2 * C:3 * C] @ W_proj).astype(np.float32)

    out = np.empty((B, T, C), dtype=np.float32)
    for b_i in range(B):
        out[b_i] = results[2 * b_i]["out"] + results[2 * b_i + 1]["out"]
        out[b_i] += bias_row[None, :]
    if _return_ns:
        return out, best_ns
    return out


# revision 20
# speedup vs baseline: 1.0873x; 1.0873x over previous
"""Causal self-attention (B=4, T=2048, C=1024, H=16) on 8 Trainium2 cores.

Sharding: core i handles batch b = i//2 and head-group g = i%2 (8 heads,
512 channels). The host pre-transposes x (so the kernel consumes xT and
spends no PE cycles on transposes) and pre-slices the weights per core.

Kernel structure (single fused pipeline):
  quarter Q in 0..3:  QKV matmuls for T-columns [512Q, 512Q+512)
                      interleaved (Q>=1) with attention chunk Q-1
  attention chunk 3 runs interleaved with projections of chunks 0-2
  projection of chunk 3 drains last (via the Activation engine).

Attention per (chunk c, head l): scores via f32r matmuls into PSUM,
exp on ACT -> e (bf16), causal masking applied in-place on e by GPSIMD
affine_select (only the 128-col diagonal triangles), then att@v with a
fused ones-column producing softmax denominators.

Bias folding (exact):
  - k-bias cancels in softmax (dropped).
  - v-bias contributes (b_v @ W_proj), folded into the host bias row.
  - q-bias added by DVE (tensor_scalar_add) when draining q from PSUM.
"""

import sys
from contextlib import ExitStack

import ml_dtypes
import numpy as np

BF16NP = ml_dtypes.bfloat16

sys.path.insert(0, "/opt/trn_rl_repo")

import concourse.bass as bass  # noqa: E402
import concourse.mybir as mybir  # noqa: E402
from concourse.masks import make_identity  # noqa: E402
from concourse.tile import TileContext  # noqa: E402

F32 = mybir.dt.float32
F32R = mybir.dt.float32r
BF16 = mybir.dt.bfloat16
EXP = mybir.ActivationFunctionType.Exp
IS_GE = mybir.AluOpType.is_ge

B, T, C, H, D = 4, 2048, 1024, 16, 64
NCORES = 8
HL = 8          # heads per core
CL = HL * D     # 512 local channels
KC = 8          # contraction chunks (1024 / 128)
SCALE = 1.0 / 8.0  # 1/sqrt(64)


# --------------------------------------------------------------------------
# Workaround: this walrus build accepts only ONE sync-wait per instruction.
# Tile emits several (operand deps, tail drain). Split extras onto fresh
# single-wait EventSemaphore instructions just before each offender on the
# same engine — semantics unchanged, the sequencer blocks on each in turn.
# --------------------------------------------------------------------------
def _split_multiwait_insts(nc):
    ctr = 0
    for f in nc.m.functions:
        for blk in f.blocks:
            insts = list(blk.instructions)
            new_list = []
            changed = False
            for inst in insts:
                si = inst.sync_info
                if si is not None and len(si.on_wait) > 1:
                    waits = list(si.on_wait)
                    keep_idx = len(waits) - 1
                    for i, w in enumerate(waits):
                        if w.wait_reg is not None:
                            keep_idx = i
                            break
                    for i, w in enumerate(waits):
                        if i == keep_idx:
                            continue
                        ev = mybir.InstEventSemaphore(
                            name=f"evsplit_{ctr}", ins=[], outs=[]
                        )
                        ctr += 1
                        ev.engine = inst.engine
                        ev.sync_info = mybir.SyncInfo(on_wait=[w], on_update=[])
                        new_list.append(ev)
                    inst.sync_info.on_wait = [waits[keep_idx]]
                    changed = True
                new_list.append(inst)
            if changed:
                blk.instructions = new_list


def build_bass(repeat=1):
    nc = bass.Bass("TRN2", target_bir_lowering=False, debug=False)

    xT_d = nc.dram_tensor("xT", [C, T], BF16, kind="ExternalInput")
    wqk_d = nc.dram_tensor("wqk", [C, 2 * CL], BF16, kind="ExternalInput")
    wv_d = nc.dram_tensor("wv", [C, CL], BF16, kind="ExternalInput")
    bq_d = nc.dram_tensor("bq", [4, 128, 1], F32, kind="ExternalInput")
    wp_d = nc.dram_tensor("wp", [CL, C], BF16, kind="ExternalInput")
    out_d = nc.dram_tensor("out", [T, C], F32, kind="ExternalOutput")
    scr_d = nc.dram_tensor("scr", [4 * HL, 512], F32)  # recip bounce buffer

    with TileContext(nc) as tc:
        for _rep in range(repeat):
            _emit_body(nc, tc, xT_d, wqk_d, wv_d, bq_d, wp_d, out_d, scr_d)

    _split_multiwait_insts(nc)
    return nc


def _emit_body(nc, tc, xT_d, wqk_d, wv_d, bq_d, wp_d, out_d, scr_d):
    with ExitStack() as ctx:
        const = ctx.enter_context(tc.tile_pool(name="const", bufs=1))
        big = ctx.enter_context(tc.tile_pool(name="big", bufs=1))

        # persistent tensors
        qkT = big.tile([128, KC, T], F32R)      # dim1: 0-3 q m-tiles, 4-7 k
        vaug = big.tile([128, 16, HL * 65], BF16)  # per T-block: 8x(64 v + 1)
        yT = big.tile([128, 4, T], BF16)        # local channels x T

        bq_sb = const.tile([128, 4, 1], F32)
        nc.sync.dma_start(out=bq_sb, in_=bq_d.ap().rearrange("a p o -> p a o"))

        e_ctx = ctx.enter_context(tc.tile_pool(name="epool", bufs=4))
        rc_pool = ctx.enter_context(tc.tile_pool(name="rcpool", bufs=3))
        ynat_pool = ctx.enter_context(tc.tile_pool(name="ynat", bufs=2))
        out_pool = ctx.enter_context(tc.tile_pool(name="opool", bufs=2))
        qk_ps = ctx.enter_context(
            tc.tile_pool(name="qkps", bufs=2, space="PSUM")
        )
        y_ps = ctx.enter_context(tc.tile_pool(name="yps", bufs=2, space="PSUM"))

        idn = const.tile([128, 128], F32)
        make_identity(nc, idn)
        idn_bf = const.tile([128, 128], BF16)
        nc.vector.tensor_copy(out=idn_bf, in_=idn)
        # additive causal masks: 0 on/above valid region, -30000 below
        tri_neg = const.tile([128, 128], BF16)
        nc.vector.memset(tri_neg, 0.0)
        nc.gpsimd.affine_select(
            out=tri_neg, in_=tri_neg, pattern=[[1, 128]],
            compare_op=IS_GE, fill=-30000.0, base=0, channel_multiplier=-1,
        )
        m2_neg = const.tile([128, 256], BF16)
        nc.vector.memset(m2_neg, 0.0)
        nc.gpsimd.affine_select(
            out=m2_neg, in_=m2_neg, pattern=[[1, 256]],
            compare_op=IS_GE, fill=-30000.0, base=-128, channel_multiplier=-1,
        )

        # ------------------------------------------------------------------
        # attention emission units for one (chunk c, head l) group
        # ------------------------------------------------------------------
        def att_group_units(c, l, ynat):
            """Yield emit-closures: scores/exp/mask for pair pj, then avs
            (software-pipelined one pair behind), then finalize."""
            row = (l % 2) * 64
            qtile = l // 2
            ktile = 4 + l // 2
            npairs = 2 * c + 2
            state = {"yps": None, "pairs": [None] * npairs}

            def make_scores(pj):
                def emit():
                    if state["yps"] is None:
                        state["yps"] = y_ps.tile([128, 4, 65], F32, tag="yps", name="yps")
                    j0 = 2 * pj - 4 * c
                    pqk = qk_ps.tile([128, 1024], F32, tag="qk")
                    e = e_ctx.tile([128, 1024], BF16, tag="e")
                    # per-half write windows (multiples of 128, >=256 wide
                    # except the 384 case, to keep f32r at full rate)
                    if j0 == 0:
                        los = (0, 128)
                    elif j0 == 2:
                        los = (256, 256)
                    else:
                        los = (0, 0)
                    # diagonal slices that need an additive causal mask,
                    # injected by the PE itself (identity @ mask-const)
                    masked = {}
                    if j0 == 0:
                        masked = {0: (0, tri_neg), 1: (128, tri_neg)}
                    elif j0 == 2:
                        masked = {0: (256, tri_neg), 1: (256, m2_neg)}
                    for h in range(2):
                        tk = 2 * pj + h
                        lo = los[h]
                        has_mask = h in masked
                        nc.tensor.matmul(
                            out=pqk[:, h * 512 + lo:(h + 1) * 512],
                            lhsT=qkT[row:row + 64, ktile,
                                     tk * 128:(tk + 1) * 128],
                            rhs=qkT[row:row + 64, qtile,
                                    c * 512 + lo:(c + 1) * 512],
                            start=True,
                            stop=not has_mask,
                        )
                        if has_mask:
                            off, mconst = masked[h]
                            sl = slice(h * 512 + off, h * 512 + off + mconst.shape[-1])
                            nc.tensor.matmul(
                                out=pqk[:, sl],
                                lhsT=idn_bf,
                                rhs=mconst,
                                start=False,
                                stop=True,
                                skip_group_check=True,
                            )
                    # single exp over the written union (interior stale
                    # regions are exp'd but never read by the avs)
                    lo0 = los[0]
                    nc.scalar.activation(
                        out=e[:, lo0:1024], in_=pqk[:, lo0:1024],
                        func=EXP, scale=SCALE,
                    )
                    state["pairs"][pj] = (e, los)
                return emit

            def make_av(pj):
                def emit():
                    e, los = state["pairs"][pj]
                    yps = state["yps"]
                    for h in range(2):
                        tk = 2 * pj + h
                        for qb in range(4):
                            if tk > 4 * c + qb:
                                continue  # above the causal diagonal
                            nc.tensor.matmul(
                                out=yps[:, qb, :],
                                lhsT=e[:, h * 512 + qb * 128:
                                       h * 512 + (qb + 1) * 128],
                                rhs=vaug[:, tk, l * 65:(l + 1) * 65],
                                start=(tk == 0),
                                stop=(tk == 4 * c + qb),
                            )
                    state["pairs"][pj] = None
                return emit

            def finalize():
                yps = state["yps"]
                rcp = rc_pool.tile([128, 4, 1], F32, tag="rc")
                nc.vector.reciprocal(out=rcp, in_=yps[:, :, 64:65])
                for qb in range(4):
                    nc.vector.tensor_scalar_mul(
                        out=ynat[:, qb, l, :],
                        in0=yps[:, qb, 0:64],
                        scalar1=rcp[:, qb, :],
                    )

            # software-pipelined unit order: sc0 sc1 av0 sc2 av1 ... av(n-1)
            yield make_scores(0)
            yield make_scores(1)
            for pj in range(2, npairs):
                yield make_av(pj - 2)
                yield make_scores(pj)
            yield make_av(npairs - 2)
            yield make_av(npairs - 1)
            yield finalize  # deferred by the caller's unit interleaving

        def att_chunk_units(c, trans_ref):
            ynat = ynat_pool.tile(
                [128, 4, HL, 64], F32, tag="ynat", name="ynat"
            )

            def transpose_cb(cb):
                def emit():
                    ynf = ynat.rearrange("p q l d -> p q (l d)")
                    tp = trans_ref["pool"].tile(
                        [128, 512], F32, tag=trans_ref["tag"], name="ytr"
                    )
                    for qb in range(4):
                        nc.tensor.transpose(
                            out=tp[:, qb * 128:(qb + 1) * 128],
                            in_=ynf[:, qb, cb * 128:(cb + 1) * 128],
                            identity=idn,
                        )
                    nc.vector.tensor_copy(
                        out=yT[:, cb, c * 512:(c + 1) * 512], in_=tp
                    )
                return emit

            # defer each group's finalize into the next group's stream (so
            # its DVE ops never head-of-line-block the DVE queue), and each
            # channel-block transpose by one further group
            units = []
            pending = []  # deferred units with countdowns
            for l in range(HL):
                gu = list(att_group_units(c, l, ynat))
                body, fin = gu[:-1], gu[-1]
                for i, u in enumerate(body):
                    units.append(u)
                    if pending and i == 1:
                        units.extend(pending)
                        pending = []
                pending.append(fin)
                if l % 2 == 1 and l >= 3:
                    pending.append(transpose_cb((l - 2) // 2))
            units.extend(pending)
            units.append(transpose_cb(3))
            return units

        def merge_ratio(a, b):
            """Proportionally interleave two unit lists."""
            out, bi = [], 0
            na, nb = len(a), len(b)
            for i, u in enumerate(a):
                out.append(u)
                want = (i + 1) * nb // na
                while bi < want:
                    out.append(b[bi])
                    bi += 1
            out.extend(b[bi:])
            return out

        # ------------------------------------------------------------------
        # Phase A scoped pools: x staging + qkv weights + qkv PSUM
        # ------------------------------------------------------------------
        with tc.tile_pool(name="xstage", bufs=2) as xstage, \
             tc.tile_pool(name="wqkv", bufs=1) as wqkv_pool, \
             tc.tile_pool(name="mmps", bufs=2, space="PSUM") as mm_ps:

            wqk_sb = wqkv_pool.tile([128, KC, 2 * CL], BF16)
            wv_sb = wqkv_pool.tile([128, KC, CL], BF16)
            # per-m-column-slab weight loads on the ACT queue
            for m in range(8):
                nc.scalar.dma_start(
                    out=wqk_sb[:, :, m * 128:(m + 1) * 128],
                    in_=wqk_d.ap().rearrange("(k p) c -> p k c", p=128)[
                        :, :, m * 128:(m + 1) * 128
                    ],
                )
            for m in range(4):
                nc.scalar.dma_start(
                    out=wv_sb[:, :, m * 128:(m + 1) * 128],
                    in_=wv_d.ap().rearrange("(k p) c -> p k c", p=128)[
                        :, :, m * 128:(m + 1) * 128
                    ],
                )

            xt_view = xT_d.ap().rearrange("(k p) t -> p k t", p=128)

            def qkv_tile_units(quarter):
                """12 emit-closures: 8 qk m-tiles + 4 v t-blocks, each fed by
                two half-quarter x stages."""
                halves = [None, None]

                def load_half(hh):
                    def emit():
                        xs = xstage.tile([128, KC, 256], BF16, tag="xq")
                        base = quarter * 512 + hh * 256
                        nc.sync.dma_start(
                            out=xs, in_=xt_view[:, :, base:base + 256]
                        )
                        halves[hh] = xs
                    return emit

                def qk_tile(m):
                    def emit():
                        pq = mm_ps.tile([128, 512], F32, tag="mm")
                        for hh in range(2):
                            xs = halves[hh]
                            for k in range(KC):
                                nc.tensor.matmul(
                                    out=pq[:, hh * 256:(hh + 1) * 256],
                                    lhsT=wqk_sb[:, k, m * 128:(m + 1) * 128],
                                    rhs=xs[:, k, :],
                                    start=(k == 0),
                                    stop=(k == KC - 1),
                                )
                        dst = qkT[:, m, quarter * 512:(quarter + 1) * 512]
                        if m < 4:  # q: add per-partition q-bias
                            nc.vector.tensor_scalar_add(
                                out=dst, in0=pq, scalar1=bq_sb[:, m, :]
                            )
                        else:
                            nc.vector.tensor_copy(out=dst, in_=pq)
                    return emit

                def v_tile(tt):
                    def emit():
                        hh = tt // 2
                        xs = halves[hh]
                        sub = tt % 2
                        pv = mm_ps.tile([128, 512], F32, tag="mm")
                        for k in range(KC):
                            nc.tensor.matmul(
                                out=pv,
                                lhsT=xs[:, k, sub * 128:(sub + 1) * 128],
                                rhs=wv_sb[:, k, :],
                                start=(k == 0),
                                stop=(k == KC - 1),
                            )
                        tglob = quarter * 4 + tt
                        nc.vector.memset(vaug[:, tglob, :], 1.0)
                        nc.vector.tensor_copy(
                            out=vaug[:, tglob, :].rearrange(
                                "p (h c) -> p h c", c=65
                            )[:, :, 0:64],
                            in_=pv.rearrange("p (h c) -> p h c", c=64),
                        )
                    return emit

                units = [load_half(0), load_half(1)]
                for m in range(8):  # same order as the weight-slab DMAs
                    units.append(qk_tile(m))
                for tt in range(4):
                    units.append(v_tile(tt))
                return units

            def weave(primary, filler):
                """Emit primary units, interspersing filler units evenly."""
                np_, nf = len(primary), len(filler)
                fi = 0
                for i, u in enumerate(primary):
                    u()
                    want = (i + 1) * nf // np_
                    while fi < want:
                        filler[fi]()
                        fi += 1
                while fi < nf:
                    filler[fi]()
                    fi += 1

            trans_ref = {"pool": mm_ps, "tag": "mm"}
            # win0: qkv quarter 0 alone
            for u in qkv_tile_units(0):
                u()
            # win1..3: qkv quarter Q woven with attention chunk Q-1
            for quarter in range(1, 4):
                weave(att_chunk_units(quarter - 1, trans_ref),
                      qkv_tile_units(quarter))

        # ------------------------------------------------------------------
        # win4: attention chunk 3 woven with projections of chunks 0-2;
        # then chunk-3 projections drain via ACT.
        # ------------------------------------------------------------------
        with tc.tile_pool(name="wp", bufs=1) as wp_pool, \
             tc.tile_pool(name="projps", bufs=2, space="PSUM") as proj_ps:

            wp_sb = wp_pool.tile([128, 4, C], BF16)
            for k in range(4):
                nc.scalar.dma_start(
                    out=wp_sb[:, k, :],
                    in_=wp_d.ap().rearrange("(k p) c -> p k c", p=128)[:, k, :],
                )

            def proj_units(tq, act=False):
                os_ = {}

                def piece(oc):
                    def emit():
                        if oc == 0:
                            os_["t"] = out_pool.tile([128, 1024], F32, tag="os", name="os")
                        pp = proj_ps.tile([128, 512], F32, tag="proj")
                        for k in range(4):
                            nc.tensor.matmul(
                                out=pp,
                                lhsT=yT[:, k, tq * 128:(tq + 1) * 128],
                                rhs=wp_sb[:, k, oc * 512:(oc + 1) * 512],
                                start=(k == 0),
                                stop=(k == 3),
                            )
                        dst = os_["t"][:, oc * 512:(oc + 1) * 512]
                        if act:
                            nc.scalar.copy(out=dst, in_=pp)
                        else:
                            nc.vector.tensor_copy(out=dst, in_=pp)
                        if oc == 1:
                            nc.sync.dma_start(
                                out=out_d.ap()[tq * 128:(tq + 1) * 128, :],
                                in_=os_["t"],
                            )
                    return emit

                return [piece(0), piece(1)]

            fillers = []
            for tq in range(12):  # projections of chunks 0-2
                fillers.extend(proj_units(tq))

            def weave2(primary, filler):
                np_, nf = len(primary), len(filler)
                fi = 0
                for i, u in enumerate(primary):
                    u()
                    want = (i + 1) * nf // np_
                    while fi < want:
                        filler[fi]()
                        fi += 1
                while fi < nf:
                    filler[fi]()
                    fi += 1

            trans_ref["pool"] = proj_ps
            trans_ref["tag"] = "proj"
            merged = att_chunk_units(3, trans_ref)
            # hold projections until wp has had time to load (the wp DMA can
            # only start once the qkv weight pool is freed at win3's end)
            hold = 16
            for u in merged[:hold]:
                u()
            weave2(merged[hold:], fillers)
            for tq in range(12, 16):  # chunk-3 projections drain via ACT
                for u in proj_units(tq, act=True):
                    u()


# --------------------------------------------------------------------------
# Cached PJRT execution (mirrors bass2jax.run_bass_via_pjrt but reuses the
# compiled executable across kernel() calls).
# --------------------------------------------------------------------------
_CACHE = {}


def _get_runner(repeat=1):
    key = ("runner", repeat)
    if key in _CACHE:
        return _CACHE[key]

    import jax
    from jax.sharding import Mesh, PartitionSpec
    from jax.experimental.shard_map import shard_map
    from concourse import bass2jax

    nc = build_bass(repeat=repeat)
    bass2jax.install_neuronx_cc_hook()

    partition_name = (
        nc.partition_id_tensor.name if nc.partition_id_tensor else None
    )
    in_names, out_names, out_avals, zero_shapes = [], [], [], []
    for alloc in nc.m.functions[0].allocations:
        if not isinstance(alloc, mybir.MemoryLocationSet):
            continue
        name = alloc.memorylocations[0].name
        if alloc.kind == "ExternalInput":
            if name != partition_name:
                in_names.append(name)
        elif alloc.kind == "ExternalOutput":
            shape = tuple(alloc.tensor_shape)
            dtype = mybir.dt.np(alloc.dtype)
            out_names.append(name)
            out_avals.append(jax.core.ShapedArray(shape, dtype))
            zero_shapes.append((shape, dtype))
    n_params = len(in_names)
    n_outs = len(out_avals)
    all_in_names = list(in_names) + list(out_names)
    if partition_name is not None:
        all_in_names.append(partition_name)

    def _body(*args):
        operands = list(args)
        if partition_name is not None:
            operands.append(bass2jax.partition_id_tensor())
        outs = bass2jax._bass_exec_p.bind(
            *operands,
            out_avals=tuple(out_avals),
            in_names=tuple(all_in_names),
            out_names=tuple(out_names),
            lowering_input_output_aliases=(),
            sim_require_finite=True,
            sim_require_nnan=True,
            nc=nc,
        )
        return tuple(outs)

    devices = jax.devices()[:NCORES]
    mesh = Mesh(np.asarray(devices), ("core",))
    in_specs = (PartitionSpec("core"),) * (n_params + n_outs)
    out_specs = (PartitionSpec("core"),) * n_outs
    donate = tuple(range(n_params, n_params + n_outs))
    sharded = jax.jit(
        shard_map(
            _body, mesh=mesh, in_specs=in_specs, out_specs=out_specs,
            check_rep=False,
        ),
        donate_argnums=donate,
        keep_unused=True,
    )

    runner = {
        "sharded": sharded,
        "in_names": in_names,
        "out_names": out_names,
        "zero_shapes": zero_shapes,
        "n_params": n_params,
        "mesh": mesh,
    }
    _CACHE[key] = runner
    return runner


def _make_core_inputs(x, W_attn, b_attn, W_proj):
    """Per-core input dicts (core i: batch i//2, head-group i%2)."""
    x = np.ascontiguousarray(x, dtype=np.float32)
    W_attn = np.ascontiguousarray(W_attn, dtype=np.float32)
    b_attn = np.ascontiguousarray(b_attn, dtype=np.float32)
    W_proj = np.ascontiguousarray(W_proj, dtype=np.float32)

    per_group = []
    for g in range(2):
        s = g * CL
        wqk = np.ascontiguousarray(
            np.concatenate(
                [W_attn[:, s:s + CL], W_attn[:, C + s:C + s + CL]], axis=1
            ).astype(BF16NP)
        )
        wv = np.ascontiguousarray(
            W_attn[:, 2 * C + s:2 * C + s + CL].astype(BF16NP)
        )
        bq = np.ascontiguousarray(b_attn[s:s + CL].reshape(4, 128, 1))
        wp = np.ascontiguousarray(W_proj[s:s + CL, :].astype(BF16NP))
        per_group.append((wqk, wv, bq, wp))

    xts = [np.ascontiguousarray(x[b].T.astype(BF16NP)) for b in range(B)]

    in_maps = []
    for core in range(NCORES):
        b_i, g = core // 2, core % 2
        wqk, wv, bq, wp = per_group[g]
        in_maps.append(
            {"xT": xts[b_i], "wqk": wqk, "wv": wv, "bq": bq, "wp": wp}
        )
    return in_maps


def run_cores(in_maps, timing_reps=0, repeat=1):
    """Run the SPMD kernel. Returns (list of per-core output dicts, best_ns).

    timing_reps > 0 additionally re-executes the cached executable on
    device-resident inputs and reports the best wall-clock per call in ns.
    """
    import jax, time

    r = _get_runner(repeat=repeat)
    per_core = [
        [np.asarray(m[name]) for name in r["in_names"]] for m in in_maps
    ]
    concat_in = [
        np.concatenate([per_core[c][i] for c in range(NCORES)], axis=0)
        for i in range(len(r["in_names"]))
    ]
    def zeros():
        return [
            np.zeros((NCORES * s[0], *s[1:]), dt) for (s, dt) in r["zero_shapes"]
        ]

    out_arrs = r["sharded"](*concat_in, *zeros())
    outs_np = [np.asarray(a) for a in out_arrs]

    best_ns = None
    if timing_reps > 0:
        from jax.sharding import NamedSharding, PartitionSpec

        shard = NamedSharding(r["mesh"], PartitionSpec("core"))
        dev_in = [jax.device_put(a, shard) for a in concat_in]
        for a in dev_in:
            a.block_until_ready()
        # pre-stage one donated zero-set per timed call (donation consumes them)
        zsets = []
        for _ in range(timing_reps + 1):
            zs = [jax.device_put(z, shard) for z in zeros()]
            for a in zs:
                a.block_until_ready()
            zsets.append(zs)
        res = r["sharded"](*dev_in, *zsets[0])  # warm
        for a in res:
            a.block_until_ready()
        times = []
        for i in range(timing_reps):
            t0 = time.perf_counter()
            res = r["sharded"](*dev_in, *zsets[i + 1])
            for a in res:
                a.block_until_ready()
            t1 = time.perf_counter()
            times.append(t1 - t0)
        best_ns = int(min(times) * 1e9)

    results = []
    for c in range(NCORES):
        m = {}
        for i, name in enumerate(r["out_names"]):
            full = outs_np[i]
            shape = r["zero_shapes"][i][0]
            m[name] = full.reshape(NCORES, *shape)[c]
        results.append(m)
    return results, best_ns


def kernel(x, W_attn, b_attn, W_proj, b_proj, _timing_reps=0, _return_ns=False):
    x = np.asarray(x, dtype=np.float32)
    W_attn = np.asarray(W_attn, dtype=np.float32)
    b_attn = np.asarray(b_attn, dtype=np.float32)
    W_proj = np.asarray(W_proj, dtype=np.float32)
    b_proj = np.asarray(b_proj, dtype=np.float32)

    in_maps = _make_core_inputs(x, W_attn, b_attn, W_proj)
    results, best_ns = run_cores(in_maps, timing_reps=_timing_reps)

    # v-bias contributes a constant row through the projection
    bias_row = (b_proj + b_attn[2 * C:3 * C] @ W_proj).astype(np.float32)

    out = np.empty((B, T, C), dtype=np.float32)
    for b_i in range(B):
        out[b_i] = results[2 * b_i]["out"] + results[2 * b_i + 1]["out"]
        out[b_i] += bias_row[None, :]
    if _return_ns:
        return out, best_ns
    return out
